# revision 16
# baseline (speedup 1.0000x reference)
"""Trainium2 Bass kernel for NodeUpdateNetwork-style GNN message passing.

out = relu(BN((x + ((sim - dsim) @ x) / N) @ W.T))  with sync-BN over (B, N).

Sharding: data-parallel over batch across 8 NeuronCores (2 batches/core);
W/gamma/beta replicated; BN statistics all-reduced across cores in-kernel.

v2 pipeline (per core, per pass):
  - edge stream: ONE 4 MB HWDGE DMA per 256-row chunk carries BOTH the sim
    and dsim stripes ([128, 2(s), 2(r), N] fp32), keeping the SP queue at
    line rate with minimal per-DMA overhead.
  - DVE: diff = sim - dsim (bf16 out), one instruction per chunk.
  - PE: transpose 128x128 diff tiles (identity matmul, bf16) -> PSUM,
    copy to SBUF (DVE/ACT alternate), then aggT[f,i] += xn[j,f]^T dT[j,i].
  - yT = aggT + xT (DVE); zT = W @ yT (PE); BN partial sums per chunk.
  - sync-BN AllReduce of [f, 2] stats launched at end of the stream phase.
  - TAIL (BN apply + untranspose + store) is software-pipelined: pass p's
    tail is emitted AFTER pass p+1's stream phase, so the collective+apply
    never stall the edge-stream engines; tail DMAs ride gpsimd/ACT queues,
    never the SP edge-stream queue.
"""

import sys

if "/opt/trn_rl_repo" not in sys.path:
    sys.path.insert(0, "/opt/trn_rl_repo")

import numpy as np
import ml_dtypes

import concourse.bacc as bacc
import concourse.mybir as mybir
import concourse.tile as tile
from concourse.bass_utils import run_bass_kernel_spmd

N_CORES = 8
B, N, F = 16, 2048, 64
B_PC = B // N_CORES
BN_EPS = 1e-5
BF16 = mybir.dt.bfloat16
F32 = mybir.dt.float32


def build_nc(
    n_cores=N_CORES, b_pc=B_PC, n=N, f=F, b_total=None, reps=1, mode="full"
):
    """Build the per-core Bass program (same program on every core).

    reps > 1 unrolls the whole computation multiple times (for timing-slope
    measurements: HW time per pass = (t(reps=R) - t(reps=1)) / (R - 1)).
    mode: "full" | "nocc" (collective replaced by local dram copy, timing
    only) | "dmaonly" (edge stream loads only, timing only).
    """
    assert f == 64
    if b_total is None:
        b_total = n_cores * b_pc
    NT = n // 128                      # number of 128-wide j tiles
    CH = 256                           # chunk height (i rows per stream DMA)
    RB = CH // 128                     # 128-row blocks per chunk
    NCH = n // CH                      # chunks per batch
    inv_count = 1.0 / (b_total * n)

    nc = bacc.Bacc(
        "TRN2", target_bir_lowering=False, debug=False, num_devices=n_cores
    )

    edge = nc.dram_tensor("edge", [b_pc, 2, n, n], F32, kind="ExternalInput").ap()
    xt = nc.dram_tensor("xt", [b_pc, f, n], F32, kind="ExternalInput").ap()
    xn = nc.dram_tensor("xn", [b_pc, n, f], BF16, kind="ExternalInput").ap()
    wt = nc.dram_tensor("wt", [f, f], F32, kind="ExternalInput").ap()
    gamma = nc.dram_tensor("gamma", [f, 1], F32, kind="ExternalInput").ap()
    beta = nc.dram_tensor("beta", [f, 1], F32, kind="ExternalInput").ap()
    i128 = nc.dram_tensor("i128", [128, 128], BF16, kind="ExternalInput").ap()
    i64 = nc.dram_tensor("i64", [f, f], BF16, kind="ExternalInput").ap()
    out = nc.dram_tensor("out", [b_pc, n, f], F32, kind="ExternalOutput").ap()

    with tile.TileContext(nc) as tc:
        with (
            tc.tile_pool(name="const", bufs=1) as cpool,
            tc.tile_pool(name="xtp", bufs=2) as xtpool,
            tc.tile_pool(name="xnp", bufs=2) as xnpool,
            tc.tile_pool(name="zq", bufs=2 * b_pc) as zqpool,
            tc.tile_pool(name="stats", bufs=2) as stpool,
            tc.tile_pool(name="stream", bufs=3) as spool,
            tc.tile_pool(name="diff", bufs=2) as dfpool,
            tc.tile_pool(name="dT", bufs=2) as dTpool,
            tc.tile_pool(name="yT", bufs=2) as yTpool,
            tc.tile_pool(name="sq", bufs=2) as sqpool,
            tc.tile_pool(name="zr", bufs=2) as zrpool,
            tc.tile_pool(name="outp", bufs=2) as outpool,
            tc.tile_pool(name="tp_ps", bufs=2, space="PSUM") as tppool,
            tc.tile_pool(name="ag_ps", bufs=2, space="PSUM") as agpool,
            tc.tile_pool(name="zt_ps", bufs=2, space="PSUM") as ztpool,
            tc.tile_pool(name="bp_ps", bufs=2, space="PSUM") as bppool,
            tc.tile_pool(name="dram", bufs=4, space="DRAM") as drpool,
        ):
            # --- constants (ACT queue; SP stays dedicated to edge stream) ---
            i128_sb = cpool.tile([128, 128], BF16)
            nc.scalar.dma_start(i128_sb[:], i128[:])
            i64_sb = cpool.tile([f, f], BF16)
            nc.scalar.dma_start(i64_sb[:], i64[:])
            wt_sb = cpool.tile([f, f], F32)
            nc.scalar.dma_start(wt_sb[:], wt[:])
            gamma_sb = cpool.tile([f, 1], F32)
            nc.scalar.dma_start(gamma_sb[:], gamma[:])
            beta_sb = cpool.tile([f, 1], F32)
            nc.scalar.dma_start(beta_sb[:], beta[:])

            def dma_only_pass(cast=False):
                # dummy consumer so bacc/walrus DCE keeps the loads
                dum = cpool.tile([128, 2], F32, tag="dum")
                for b in range(b_pc):
                    for c in range(NCH):
                        i0 = c * CH
                        if cast:
                            st_sb = spool.tile([128, 2, RB * n], BF16, tag="stc")
                            nc.gpsimd.dma_start(
                                st_sb[:],
                                edge[b, :, i0 : i0 + CH, :].rearrange(
                                    "s (p r) n -> p s (r n)", r=RB
                                ),
                            )
                        else:
                            st_sb = spool.tile([128, 2, RB * n], F32, tag="st")
                            nc.sync.dma_start(
                                st_sb[:],
                                edge[b, :, i0 : i0 + CH, :].rearrange(
                                    "s (p r) n -> p s (r n)", r=RB
                                ),
                            )
                        nc.vector.reduce_sum(
                            dum[:, 0:1], st_sb[:, 0, 0:4],
                            axis=mybir.AxisListType.X,
                        )
                nc.gpsimd.dma_start(out[0, 0:128, 0:2], dum[:])

            def stream_pass():
                zq_tiles = []
                stats_sb = stpool.tile([f, b_pc * NCH, 2], F32, tag="stats")
                for b in range(b_pc):
                    # --- per-batch node features (ACT queue) ---
                    xt_sb = xtpool.tile([f, n], F32, tag="xt")
                    nc.scalar.dma_start(xt_sb[:], xt[b])
                    xn_sb = xnpool.tile([128, NT, f], BF16, tag="xn")
                    nc.scalar.dma_start(
                        xn_sb[:], xn[b].rearrange("(t p) f -> p t f", p=128)
                    )
                    zq_sb = zqpool.tile([f, n], BF16, tag="zq")
                    zq_tiles.append(zq_sb)

                    for c in range(NCH):
                        i0 = c * CH
                        # --- ONE 4MB DMA: sim+dsim stripes for CH rows.
                        # Row interleave: chunk row (2p + r) lands on
                        # partition p, slot r — a 3-dim access pattern with
                        # 16KB contiguous reads per (partition, plane).
                        # Downstream free-axis positions within the chunk are
                        # pos = r*128 + q  <->  global row i0 + 2q + r; the
                        # host pre-permutes xt and the out store un-permutes.
                        st_sb = spool.tile([128, 2, RB * n], F32, tag="st")
                        nc.sync.dma_start(
                            st_sb[:],
                            edge[b, :, i0 : i0 + CH, :].rearrange(
                                "s (p r) n -> p s (r n)", r=RB
                            ),
                        )
                        # --- diff = sim - dsim (bf16), one DVE op ---
                        diff = dfpool.tile([128, RB * n], BF16, tag="diff")
                        nc.vector.tensor_sub(
                            diff[:], st_sb[:, 0], st_sb[:, 1]
                        )

                        # --- transpose diff tiles: dT[j, i] = diff[i, j] ---
                        dT = dTpool.tile([128, NT, CH], BF16, tag="dT")
                        for jt in range(NT):
                            tp = tppool.tile([128, CH], BF16, tag="tp")
                            for r in range(RB):
                                nc.tensor.transpose(
                                    tp[:, r * 128 : (r + 1) * 128],
                                    diff[
                                        :,
                                        r * n + jt * 128 : r * n + (jt + 1) * 128,
                                    ],
                                    i128_sb[:],
                                )
                            if jt % 2 == 0:
                                nc.vector.tensor_copy(dT[:, jt, :], tp[:])
                            else:
                                nc.scalar.copy(dT[:, jt, :], tp[:])

                        # --- aggT[f, i] = sum_j (x/N)[j, f] * diff[i, j] ---
                        agg = agpool.tile([f, CH], F32, tag="agg")
                        for jt in range(NT):
                            nc.tensor.matmul(
                                agg[:],
                                xn_sb[:, jt, :],
                                dT[:, jt, :],
                                start=(jt == 0),
                                stop=(jt == NT - 1),
                            )

                        # --- yT = aggT + xT ; zT = W @ yT ---
                        yT = yTpool.tile([f, CH], F32, tag="yT")
                        nc.vector.tensor_add(
                            yT[:], agg[:], xt_sb[:, i0 : i0 + CH]
                        )
                        zT = ztpool.tile([f, CH], F32, tag="zT")
                        nc.tensor.matmul(
                            zT[:], wt_sb[:], yT[:], start=True, stop=True
                        )

                        # stash z and accumulate BN partial sums
                        gi = b * NCH + c
                        nc.scalar.copy(zq_sb[:, i0 : i0 + CH], zT[:])
                        nc.vector.reduce_sum(
                            stats_sb[:, gi, 0:1], zT[:],
                            axis=mybir.AxisListType.X,
                        )
                        sq = sqpool.tile([f, CH], F32, tag="sq")
                        nc.scalar.activation(
                            sq[:],
                            zT[:],
                            mybir.ActivationFunctionType.Square,
                            accum_out=stats_sb[:, gi, 1:2],
                        )

                # --- local stats -> launch sync-BN all-reduce (gpsimd) ---
                stats_loc = stpool.tile([f, 2], F32, tag="loc")
                nc.vector.reduce_sum(
                    stats_loc[:],
                    stats_sb[:].rearrange("p g s -> p s g"),
                    axis=mybir.AxisListType.X,
                )
                cc_in = drpool.tile([f, 2], F32, tag="cc_in")
                cc_out = drpool.tile([f, 2], F32, tag="cc_out")
                nc.gpsimd.dma_start(cc_in[:], stats_loc[:])
                if mode == "nocc":
                    nc.gpsimd.dma_start(cc_out[:], cc_in[:])
                else:
                    nc.gpsimd.collective_compute(
                        "AllReduce",
                        mybir.AluOpType.add,
                        replica_groups=[list(range(n_cores))],
                        ins=[cc_in.opt()],
                        outs=[cc_out.opt()],
                    )
                return {"zq": zq_tiles, "cc_out": cc_out}

            def tail_pass(st):
                stats_tot = stpool.tile([f, 2], F32, tag="tot")
                nc.gpsimd.dma_start(stats_tot[:], st["cc_out"][:])

                # --- mean/var -> scale/shift ---
                sc_sb = stpool.tile([f, 12], F32, tag="sc")
                mean = sc_sb[:, 0:1]
                es2 = sc_sb[:, 1:2]
                msq = sc_sb[:, 2:3]
                var = sc_sb[:, 3:4]
                std = sc_sb[:, 4:5]
                rstd = sc_sb[:, 5:6]
                scl = sc_sb[:, 6:7]
                tmp = sc_sb[:, 7:8]
                shf = sc_sb[:, 8:9]
                varp = sc_sb[:, 9:10]
                nc.vector.tensor_scalar_mul(mean, stats_tot[:, 0:1], inv_count)
                nc.vector.tensor_scalar_mul(es2, stats_tot[:, 1:2], inv_count)
                nc.vector.tensor_mul(msq, mean, mean)
                nc.vector.tensor_sub(var, es2, msq)
                nc.vector.tensor_scalar_add(varp, var, BN_EPS)
                nc.scalar.activation(std, varp, mybir.ActivationFunctionType.Sqrt)
                nc.vector.reciprocal(rstd, std)
                nc.vector.tensor_mul(scl, gamma_sb[:], rstd)
                nc.vector.tensor_mul(tmp, mean, scl)
                nc.vector.tensor_sub(shf, beta_sb[:], tmp)

                # --- apply BN+ReLU, untranspose, store ---
                for b in range(b_pc):
                    zr_sb = zrpool.tile([f, n], BF16, tag="zr")
                    nc.scalar.activation(
                        zr_sb[:],
                        st["zq"][b][:],
                        mybir.ActivationFunctionType.Relu,
                        bias=shf,
                        scale=scl,
                    )
                    out_sb = outpool.tile([128, NCH, RB * f], F32, tag="out")
                    for ct in range(NT):
                        bp = bppool.tile([128, f], BF16, tag="bp")
                        nc.tensor.transpose(
                            bp[:], zr_sb[:, ct * 128 : (ct + 1) * 128], i64_sb[:]
                        )
                        nc.vector.tensor_copy(
                            out_sb[
                                :, ct // RB, (ct % RB) * f : (ct % RB + 1) * f
                            ],
                            bp[:],
                        )
                    nc.gpsimd.dma_start(
                        out[b].rearrange("(c q r) f -> q c (r f)", q=128, r=RB),
                        out_sb[:],
                    )

            if mode in ("dmaonly", "dmacast"):
                for _ in range(reps):
                    dma_only_pass(cast=(mode == "dmacast"))
            else:
                prev = None
                for _ in range(reps):
                    cur = stream_pass()
                    if prev is not None:
                        tail_pass(prev)
                    prev = cur
                tail_pass(prev)

    nc.compile()
    return nc


def make_in_maps(node_feats, edge_feats, W, gamma, beta, n_cores=N_CORES):
    b, n, f = node_feats.shape
    b_pc = b // n_cores
    node_feats = np.asarray(node_feats, dtype=np.float32)
    edge_feats = np.asarray(edge_feats, dtype=np.float32)
    wt = np.ascontiguousarray(np.asarray(W, dtype=np.float32).T)
    gamma = np.asarray(gamma, dtype=np.float32).reshape(f, 1)
    beta = np.asarray(beta, dtype=np.float32).reshape(f, 1)
    i128 = np.eye(128, dtype=np.float32).astype(ml_dtypes.bfloat16)
    i64 = np.eye(f, dtype=np.float32).astype(ml_dtypes.bfloat16)
    in_maps = []
    # xt columns are pre-permuted to the kernel's row-interleaved chunk
    # order: chunk-local position r*128 + q holds global row 2q + r.
    CH, RB = 256, 2
    nch = n // CH
    for c in range(n_cores):
        sl = slice(c * b_pc, (c + 1) * b_pc)
        xs = node_feats[sl]
        xtb = xs.transpose(0, 2, 1)  # [b_pc, f, n]
        xtp = (
            xtb.reshape(b_pc, f, nch, CH // RB, RB)
            .swapaxes(3, 4)
            .reshape(b_pc, f, n)
        )
        in_maps.append(
            {
                "edge": edge_feats[sl],
                "xt": np.ascontiguousarray(xtp),
                "xn": (xs / np.float32(n)).astype(ml_dtypes.bfloat16),
                "wt": wt,
                "gamma": gamma,
                "beta": beta,
                "i128": i128,
                "i64": i64,
            }
        )
    return in_maps


_NC_CACHE = {}


def _get_nc(key=(N_CORES, B_PC, N, F)):
    if key not in _NC_CACHE:
        _NC_CACHE[key] = build_nc(*key)
    return _NC_CACHE[key]


def kernel(node_feats, edge_feats, W, gamma, beta):
    node_feats = np.asarray(node_feats)
    edge_feats = np.asarray(edge_feats)
    b, n, f = node_feats.shape
    n_cores = N_CORES
    b_pc = b // n_cores
    nc = _get_nc((n_cores, b_pc, n, f))
    in_maps = make_in_maps(node_feats, edge_feats, W, gamma, beta, n_cores)
    res = run_bass_kernel_spmd(nc, in_maps, list(range(n_cores)))
    outs = [res.results[c]["out"] for c in range(n_cores)]
    return np.concatenate(outs, axis=0).astype(np.float32)


# revision 17
# speedup vs baseline: 1.1896x; 1.1896x over previous
"""Trainium2 Bass kernel for NodeUpdateNetwork-style GNN message passing.

out = relu(BN((x + ((sim - dsim) @ x) / N) @ W.T))  with sync-BN over (B, N).

Sharding: data-parallel over batch across 8 NeuronCores (2 batches/core);
W/gamma/beta replicated; BN statistics all-reduced across cores in-kernel.

v2 pipeline (per core, per pass):
  - edge stream: ONE 4 MB HWDGE DMA per 256-row chunk carries BOTH the sim
    and dsim stripes ([128, 2(s), 2(r), N] fp32), keeping the SP queue at
    line rate with minimal per-DMA overhead.
  - DVE: diff = sim - dsim (bf16 out), one instruction per chunk.
  - PE: transpose 128x128 diff tiles (identity matmul, bf16) -> PSUM,
    copy to SBUF (DVE/ACT alternate), then aggT[f,i] += xn[j,f]^T dT[j,i].
  - yT = aggT + xT (DVE); zT = W @ yT (PE); BN partial sums per chunk.
  - sync-BN AllReduce of [f, 2] stats launched at end of the stream phase.
  - TAIL (BN apply + untranspose + store) is software-pipelined: pass p's
    tail is emitted AFTER pass p+1's stream phase, so the collective+apply
    never stall the edge-stream engines; tail DMAs ride gpsimd/ACT queues,
    never the SP edge-stream queue.
"""

import sys

if "/opt/trn_rl_repo" not in sys.path:
    sys.path.insert(0, "/opt/trn_rl_repo")

import numpy as np
import ml_dtypes

import concourse.bacc as bacc
import concourse.mybir as mybir
import concourse.tile as tile
from concourse.bass_utils import run_bass_kernel_spmd

N_CORES = 8
B, N, F = 16, 2048, 64
B_PC = B // N_CORES
BN_EPS = 1e-5
BF16 = mybir.dt.bfloat16
F32 = mybir.dt.float32


def build_nc(
    n_cores=N_CORES, b_pc=B_PC, n=N, f=F, b_total=None, reps=1, mode="full"
):
    """Build the per-core Bass program (same program on every core).

    reps > 1 unrolls the whole computation multiple times (for timing-slope
    measurements: HW time per pass = (t(reps=R) - t(reps=1)) / (R - 1)).
    mode: "full" | "nocc" (collective replaced by local dram copy, timing
    only) | "dmaonly" (edge stream loads only, timing only).
    """
    assert f == 64
    if b_total is None:
        b_total = n_cores * b_pc
    NT = n // 128                      # number of 128-wide j tiles
    CH = 256                           # chunk height (i rows per stream DMA)
    RB = CH // 128                     # 128-row blocks per chunk
    NCH = n // CH                      # chunks per batch
    inv_count = 1.0 / (b_total * n)

    nc = bacc.Bacc(
        "TRN2", target_bir_lowering=False, debug=False, num_devices=n_cores
    )

    edge = nc.dram_tensor("edge", [b_pc, 2, n, n], F32, kind="ExternalInput").ap()
    xt = nc.dram_tensor("xt", [b_pc, f, n], F32, kind="ExternalInput").ap()
    xn = nc.dram_tensor("xn", [b_pc, n, f], BF16, kind="ExternalInput").ap()
    wt = nc.dram_tensor("wt", [f, f], F32, kind="ExternalInput").ap()
    gamma = nc.dram_tensor("gamma", [f, 1], F32, kind="ExternalInput").ap()
    beta = nc.dram_tensor("beta", [f, 1], F32, kind="ExternalInput").ap()
    i128 = nc.dram_tensor("i128", [128, 128], BF16, kind="ExternalInput").ap()
    i64 = nc.dram_tensor("i64", [f, f], BF16, kind="ExternalInput").ap()
    out = nc.dram_tensor("out", [b_pc, n, f], F32, kind="ExternalOutput").ap()

    with tile.TileContext(nc) as tc:
        with (
            tc.tile_pool(name="const", bufs=1) as cpool,
            tc.tile_pool(name="xtp", bufs=2) as xtpool,
            tc.tile_pool(name="xnp", bufs=2) as xnpool,
            tc.tile_pool(name="zq", bufs=2 * b_pc) as zqpool,
            tc.tile_pool(name="stats", bufs=2) as stpool,
            tc.tile_pool(name="bnsc", bufs=2) as bnpool,
            tc.tile_pool(name="stream", bufs=3) as spool,
            tc.tile_pool(name="diff", bufs=2) as dfpool,
            tc.tile_pool(name="dT", bufs=2) as dTpool,
            tc.tile_pool(name="yT", bufs=2) as yTpool,
            tc.tile_pool(name="sq", bufs=2) as sqpool,
            tc.tile_pool(name="zr", bufs=2) as zrpool,
            tc.tile_pool(name="outp", bufs=2) as outpool,
            tc.tile_pool(name="tp_ps", bufs=2, space="PSUM") as tppool,
            tc.tile_pool(name="ag_ps", bufs=2, space="PSUM") as agpool,
            tc.tile_pool(name="zt_ps", bufs=2, space="PSUM") as ztpool,
            tc.tile_pool(name="bp_ps", bufs=2, space="PSUM") as bppool,
            tc.tile_pool(name="dram", bufs=4, space="DRAM") as drpool,
        ):
            # --- constants (ACT queue; SP stays dedicated to edge stream) ---
            i128_sb = cpool.tile([128, 128], BF16)
            nc.scalar.dma_start(i128_sb[:], i128[:])
            i64_sb = cpool.tile([f, f], BF16)
            nc.scalar.dma_start(i64_sb[:], i64[:])
            wt_sb = cpool.tile([f, f], F32)
            nc.scalar.dma_start(wt_sb[:], wt[:])
            gamma_sb = cpool.tile([f, 1], F32)
            nc.scalar.dma_start(gamma_sb[:], gamma[:])
            beta_sb = cpool.tile([f, 1], F32)
            nc.scalar.dma_start(beta_sb[:], beta[:])

            def dma_only_pass(cast=False):
                # dummy consumer so bacc/walrus DCE keeps the loads
                dum = cpool.tile([128, 2], F32, tag="dum")
                for b in range(b_pc):
                    for c in range(NCH):
                        i0 = c * CH
                        if cast:
                            st_sb = spool.tile([128, 2, RB * n], BF16, tag="stc")
                            nc.gpsimd.dma_start(
                                st_sb[:],
                                edge[b, :, i0 : i0 + CH, :].rearrange(
                                    "s (p r) n -> p s (r n)", r=RB
                                ),
                            )
                        else:
                            st_sb = spool.tile([128, 2, RB * n], F32, tag="st")
                            nc.sync.dma_start(
                                st_sb[:],
                                edge[b, :, i0 : i0 + CH, :].rearrange(
                                    "s (p r) n -> p s (r n)", r=RB
                                ),
                            )
                        nc.vector.reduce_sum(
                            dum[:, 0:1], st_sb[:, 0, 0:4],
                            axis=mybir.AxisListType.X,
                        )
                nc.gpsimd.dma_start(out[0, 0:128, 0:2], dum[:])

            def stream_pass():
                zq_tiles = []
                stats_sb = stpool.tile([f, b_pc * NCH, 2], F32, tag="stats")
                for b in range(b_pc):
                    # --- per-batch node features (ACT queue) ---
                    xt_sb = xtpool.tile([f, n], F32, tag="xt")
                    nc.scalar.dma_start(xt_sb[:], xt[b])
                    xn_sb = xnpool.tile([128, NT, f], BF16, tag="xn")
                    nc.scalar.dma_start(
                        xn_sb[:], xn[b].rearrange("(t p) f -> p t f", p=128)
                    )
                    zq_sb = zqpool.tile([f, n], BF16, tag="zq")
                    zq_tiles.append(zq_sb)

                    for c in range(NCH):
                        i0 = c * CH
                        # --- ONE 4MB DMA: sim+dsim stripes for CH rows.
                        # Row interleave: chunk row (2p + r) lands on
                        # partition p, slot r — a 3-dim access pattern with
                        # 16KB contiguous reads per (partition, plane).
                        # Downstream free-axis positions within the chunk are
                        # pos = r*128 + q  <->  global row i0 + 2q + r; the
                        # host pre-permutes xt and the out store un-permutes.
                        st_sb = spool.tile([128, 2, RB * n], F32, tag="st")
                        nc.sync.dma_start(
                            st_sb[:],
                            edge[b, :, i0 : i0 + CH, :].rearrange(
                                "s (p r) n -> p s (r n)", r=RB
                            ),
                        )
                        # --- diff = sim - dsim (bf16), one DVE op ---
                        diff = dfpool.tile([128, RB * n], BF16, tag="diff")
                        nc.vector.tensor_sub(
                            diff[:], st_sb[:, 0], st_sb[:, 1]
                        )

                        # --- transpose diff tiles: dT[j, i] = diff[i, j] ---
                        dT = dTpool.tile([128, NT, CH], BF16, tag="dT")
                        for jt in range(NT):
                            tp = tppool.tile([128, CH], BF16, tag="tp")
                            for r in range(RB):
                                nc.tensor.transpose(
                                    tp[:, r * 128 : (r + 1) * 128],
                                    diff[
                                        :,
                                        r * n + jt * 128 : r * n + (jt + 1) * 128,
                                    ],
                                    i128_sb[:],
                                )
                            if jt % 2 == 0:
                                nc.vector.tensor_copy(dT[:, jt, :], tp[:])
                            else:
                                nc.scalar.copy(dT[:, jt, :], tp[:])

                        # --- aggT[f, i] = sum_j (x/N)[j, f] * diff[i, j] ---
                        agg = agpool.tile([f, CH], F32, tag="agg")
                        for jt in range(NT):
                            nc.tensor.matmul(
                                agg[:],
                                xn_sb[:, jt, :],
                                dT[:, jt, :],
                                start=(jt == 0),
                                stop=(jt == NT - 1),
                            )

                        # --- yT = aggT + xT ; zT = W @ yT ---
                        yT = yTpool.tile([f, CH], F32, tag="yT")
                        nc.vector.tensor_add(
                            yT[:], agg[:], xt_sb[:, i0 : i0 + CH]
                        )
                        zT = ztpool.tile([f, CH], F32, tag="zT")
                        nc.tensor.matmul(
                            zT[:], wt_sb[:], yT[:], start=True, stop=True
                        )

                        # stash z and accumulate BN partial sums
                        gi = b * NCH + c
                        nc.scalar.copy(zq_sb[:, i0 : i0 + CH], zT[:])
                        nc.vector.reduce_sum(
                            stats_sb[:, gi, 0:1], zT[:],
                            axis=mybir.AxisListType.X,
                        )
                        sq = sqpool.tile([f, CH], F32, tag="sq")
                        nc.scalar.activation(
                            sq[:],
                            zT[:],
                            mybir.ActivationFunctionType.Square,
                            accum_out=stats_sb[:, gi, 1:2],
                        )

                # --- local stats -> launch sync-BN all-reduce (gpsimd) ---
                stats_loc = stpool.tile([f, 2], F32, tag="loc")
                nc.vector.reduce_sum(
                    stats_loc[:],
                    stats_sb[:].rearrange("p g s -> p s g"),
                    axis=mybir.AxisListType.X,
                )
                cc_in = drpool.tile([f, 2], F32, tag="cc_in")
                cc_out = drpool.tile([f, 2], F32, tag="cc_out")
                nc.gpsimd.dma_start(cc_in[:], stats_loc[:])
                if mode == "nocc":
                    nc.gpsimd.dma_start(cc_out[:], cc_in[:])
                else:
                    nc.gpsimd.collective_compute(
                        "AllReduce",
                        mybir.AluOpType.add,
                        replica_groups=[list(range(n_cores))],
                        ins=[cc_in.opt()],
                        outs=[cc_out.opt()],
                    )
                return {"zq": zq_tiles, "cc_out": cc_out}

            def tail_pass(st):
                stats_tot = bnpool.tile([f, 2], F32, tag="tot")
                nc.gpsimd.dma_start(stats_tot[:], st["cc_out"][:])

                # --- mean/var -> scale/shift ---
                sc_sb = bnpool.tile([f, 12], F32, tag="sc")
                mean = sc_sb[:, 0:1]
                es2 = sc_sb[:, 1:2]
                msq = sc_sb[:, 2:3]
                var = sc_sb[:, 3:4]
                std = sc_sb[:, 4:5]
                rstd = sc_sb[:, 5:6]
                scl = sc_sb[:, 6:7]
                tmp = sc_sb[:, 7:8]
                shf = sc_sb[:, 8:9]
                varp = sc_sb[:, 9:10]
                nc.vector.tensor_scalar_mul(mean, stats_tot[:, 0:1], inv_count)
                nc.vector.tensor_scalar_mul(es2, stats_tot[:, 1:2], inv_count)
                nc.vector.tensor_mul(msq, mean, mean)
                nc.vector.tensor_sub(var, es2, msq)
                nc.vector.tensor_scalar_add(varp, var, BN_EPS)
                nc.scalar.activation(std, varp, mybir.ActivationFunctionType.Sqrt)
                nc.vector.reciprocal(rstd, std)
                nc.vector.tensor_mul(scl, gamma_sb[:], rstd)
                nc.vector.tensor_mul(tmp, mean, scl)
                nc.vector.tensor_sub(shf, beta_sb[:], tmp)

                # --- apply BN+ReLU, untranspose, store ---
                for b in range(b_pc):
                    zr_sb = zrpool.tile([f, n], BF16, tag="zr")
                    nc.scalar.activation(
                        zr_sb[:],
                        st["zq"][b][:],
                        mybir.ActivationFunctionType.Relu,
                        bias=shf,
                        scale=scl,
                    )
                    out_sb = outpool.tile([128, NCH, RB * f], F32, tag="out")
                    for ct in range(NT):
                        bp = bppool.tile([128, f], BF16, tag="bp")
                        nc.tensor.transpose(
                            bp[:], zr_sb[:, ct * 128 : (ct + 1) * 128], i64_sb[:]
                        )
                        nc.vector.tensor_copy(
                            out_sb[
                                :, ct // RB, (ct % RB) * f : (ct % RB + 1) * f
                            ],
                            bp[:],
                        )
                    nc.gpsimd.dma_start(
                        out[b].rearrange("(c q r) f -> q c (r f)", q=128, r=RB),
                        out_sb[:],
                    )

            if mode in ("dmaonly", "dmacast"):
                for _ in range(reps):
                    dma_only_pass(cast=(mode == "dmacast"))
            else:
                prev = None
                for _ in range(reps):
                    cur = stream_pass()
                    if prev is not None:
                        tail_pass(prev)
                    prev = cur
                tail_pass(prev)

    nc.compile()
    return nc


def make_in_maps(node_feats, edge_feats, W, gamma, beta, n_cores=N_CORES):
    b, n, f = node_feats.shape
    b_pc = b // n_cores
    node_feats = np.asarray(node_feats, dtype=np.float32)
    edge_feats = np.asarray(edge_feats, dtype=np.float32)
    wt = np.ascontiguousarray(np.asarray(W, dtype=np.float32).T)
    gamma = np.asarray(gamma, dtype=np.float32).reshape(f, 1)
    beta = np.asarray(beta, dtype=np.float32).reshape(f, 1)
    i128 = np.eye(128, dtype=np.float32).astype(ml_dtypes.bfloat16)
    i64 = np.eye(f, dtype=np.float32).astype(ml_dtypes.bfloat16)
    in_maps = []
    # xt columns are pre-permuted to the kernel's row-interleaved chunk
    # order: chunk-local position r*128 + q holds global row 2q + r.
    CH, RB = 256, 2
    nch = n // CH
    for c in range(n_cores):
        sl = slice(c * b_pc, (c + 1) * b_pc)
        xs = node_feats[sl]
        xtb = xs.transpose(0, 2, 1)  # [b_pc, f, n]
        xtp = (
            xtb.reshape(b_pc, f, nch, CH // RB, RB)
            .swapaxes(3, 4)
            .reshape(b_pc, f, n)
        )
        in_maps.append(
            {
                "edge": edge_feats[sl],
                "xt": np.ascontiguousarray(xtp),
                "xn": (xs / np.float32(n)).astype(ml_dtypes.bfloat16),
                "wt": wt,
                "gamma": gamma,
                "beta": beta,
                "i128": i128,
                "i64": i64,
            }
        )
    return in_maps


_NC_CACHE = {}


def _get_nc(key=(N_CORES, B_PC, N, F)):
    if key not in _NC_CACHE:
        _NC_CACHE[key] = build_nc(*key)
    return _NC_CACHE[key]


def kernel(node_feats, edge_feats, W, gamma, beta):
    node_feats = np.asarray(node_feats)
    edge_feats = np.asarray(edge_feats)
    b, n, f = node_feats.shape
    n_cores = N_CORES
    b_pc = b // n_cores
    nc = _get_nc((n_cores, b_pc, n, f))
    in_maps = make_in_maps(node_feats, edge_feats, W, gamma, beta, n_cores)
    res = run_bass_kernel_spmd(nc, in_maps, list(range(n_cores)))
    outs = [res.results[c]["out"] for c in range(n_cores)]
    return np.concatenate(outs, axis=0).astype(np.float32)


# revision 29
# speedup vs baseline: 2.3530x; 1.9779x over previous
"""Trainium2 Bass kernel for NodeUpdateNetwork-style GNN message passing.

out = relu(BN((x + ((sim - dsim) @ x) / N) @ W.T))  with sync-BN over (B, N).

Sharding: data-parallel over batch across 8 NeuronCores (2 batches/core);
W/gamma/beta replicated; BN statistics all-reduced across cores in-kernel.

Pipeline (per core, per pass) — stream at the ~358 GB/s HBM-per-core floor:
  - edge stream: ONE 4 MB HWDGE DMA per 256-row chunk carries BOTH the sim
    and dsim stripes on a dedicated SP queue. A row-interleave (chunk row
    2p + r -> partition p, slot r) keeps the access pattern at 3 dims with
    16 KB contiguous reads; the out store un-permutes.
  - GPSIMD: diff = sim - dsim (bf16). The only op releasing stream buffers
    rides an engine with no PE/ACT-dependent work in its FIFO, so a lagging
    consumer can never stall the edge DMA queue.
  - PE: transpose diff tiles -> PSUM (2 j-tiles per bank); PSUM->SBUF
    copies alternate DVE/ACT. The matmul stage (aggT accumulation, the
    residual folded in via constant permutation matmuls holding 2048.0,
    zT = W @ yT, BN partial sums) is software-pipelined one chunk behind
    so PE never idles at its queue head waiting for fresh dT copies.
  - sync-BN AllReduce of [f, 2] stats launches on gpsimd at stream end;
    the TAIL (BN apply + untranspose + store) is deferred TWO passes so the
    collective rendezvous and cross-core jitter never stall any engine;
    tail DMAs ride the ACT HWDGE queue (SWDGE descriptor-ring traffic
    interferes with the edge stream's SDMA ports).
"""

import sys

if "/opt/trn_rl_repo" not in sys.path:
    sys.path.insert(0, "/opt/trn_rl_repo")

import numpy as np
import ml_dtypes

import concourse.bacc as bacc
import concourse.mybir as mybir
import concourse.tile as tile
from concourse.bass_utils import run_bass_kernel_spmd

N_CORES = 8
B, N, F = 16, 2048, 64
B_PC = B // N_CORES
BN_EPS = 1e-5
BF16 = mybir.dt.bfloat16
F32 = mybir.dt.float32


def build_nc(
    n_cores=N_CORES, b_pc=B_PC, n=N, f=F, b_total=None, reps=1, mode="full"
):
    """Build the per-core Bass program (same program on every core).

    reps > 1 unrolls the whole computation multiple times (for timing-slope
    measurements: HW time per pass = (t(reps=R) - t(reps=1)) / (R - 1)).
    mode: "full" | "nocc" (collective replaced by local dram copy, timing
    only) | "dmaonly" (edge stream loads only, timing only).
    """
    assert f == 64
    if b_total is None:
        b_total = n_cores * b_pc
    NT = n // 128                      # number of 128-wide j tiles
    CH = 256                           # chunk height (i rows per stream DMA)
    RB = CH // 128                     # 128-row blocks per chunk
    NCH = n // CH                      # chunks per batch
    inv_count = 1.0 / (b_total * n)

    nc = bacc.Bacc(
        "TRN2", target_bir_lowering=False, debug=False, num_devices=n_cores
    )

    edge = nc.dram_tensor("edge", [b_pc, 2, n, n], F32, kind="ExternalInput").ap()
    xn = nc.dram_tensor("xn", [b_pc, n, f], BF16, kind="ExternalInput").ap()
    wt = nc.dram_tensor("wt", [f, f], BF16, kind="ExternalInput").ap()
    p0 = nc.dram_tensor("p0", [128, CH], BF16, kind="ExternalInput").ap()
    p1 = nc.dram_tensor("p1", [128, CH], BF16, kind="ExternalInput").ap()
    gamma = nc.dram_tensor("gamma", [f, 1], F32, kind="ExternalInput").ap()
    beta = nc.dram_tensor("beta", [f, 1], F32, kind="ExternalInput").ap()
    i128 = nc.dram_tensor("i128", [128, 128], BF16, kind="ExternalInput").ap()
    i64 = nc.dram_tensor("i64", [f, f], BF16, kind="ExternalInput").ap()
    out = nc.dram_tensor("out", [b_pc, n, f], F32, kind="ExternalOutput").ap()

    with tile.TileContext(nc) as tc:
        with (
            tc.tile_pool(name="const", bufs=1) as cpool,
            tc.tile_pool(name="xnp", bufs=2) as xnpool,
            tc.tile_pool(name="zq", bufs=3 * b_pc) as zqpool,
            tc.tile_pool(name="stats", bufs=2) as stpool,
            tc.tile_pool(name="bnsc", bufs=2) as bnpool,
            tc.tile_pool(name="stream", bufs=3) as spool,
            tc.tile_pool(name="diff", bufs=3) as dfpool,
            tc.tile_pool(name="dT", bufs=2) as dTpool,
            tc.tile_pool(name="yT", bufs=2) as yTpool,
            tc.tile_pool(name="sq", bufs=2) as sqpool,
            tc.tile_pool(name="zr", bufs=2) as zrpool,
            tc.tile_pool(name="outp", bufs=2) as outpool,
            tc.tile_pool(name="tp_ps", bufs=3, space="PSUM") as tppool,
            tc.tile_pool(name="ag_ps", bufs=2, space="PSUM") as agpool,
            tc.tile_pool(name="zt_ps", bufs=2, space="PSUM") as ztpool,
            tc.tile_pool(name="bp_ps", bufs=1, space="PSUM") as bppool,
            tc.tile_pool(name="dram", bufs=6, space="DRAM") as drpool,
        ):
            # --- constants (ACT queue; SP stays dedicated to edge stream) ---
            i128_sb = cpool.tile([128, 128], BF16)
            nc.scalar.dma_start(i128_sb[:], i128[:])
            i64_sb = cpool.tile([f, f], BF16)
            nc.scalar.dma_start(i64_sb[:], i64[:])
            wt_sb = cpool.tile([f, f], BF16)
            nc.scalar.dma_start(wt_sb[:], wt[:])
            p0_sb = cpool.tile([128, CH], BF16)
            nc.scalar.dma_start(p0_sb[:], p0[:])
            p1_sb = cpool.tile([128, CH], BF16)
            nc.scalar.dma_start(p1_sb[:], p1[:])
            gamma_sb = cpool.tile([f, 1], F32)
            nc.scalar.dma_start(gamma_sb[:], gamma[:])
            beta_sb = cpool.tile([f, 1], F32)
            nc.scalar.dma_start(beta_sb[:], beta[:])

            def dma_only_pass(cast=False):
                # dummy consumer so bacc/walrus DCE keeps the loads
                dum = cpool.tile([128, 2], F32, tag="dum")
                for b in range(b_pc):
                    for c in range(NCH):
                        i0 = c * CH
                        if cast:
                            st_sb = spool.tile([128, 2, RB * n], BF16, tag="stc")
                            nc.gpsimd.dma_start(
                                st_sb[:],
                                edge[b, :, i0 : i0 + CH, :].rearrange(
                                    "s (p r) n -> p s (r n)", r=RB
                                ),
                            )
                        else:
                            st_sb = spool.tile([128, 2, RB * n], F32, tag="st")
                            nc.sync.dma_start(
                                st_sb[:],
                                edge[b, :, i0 : i0 + CH, :].rearrange(
                                    "s (p r) n -> p s (r n)", r=RB
                                ),
                            )
                        nc.vector.reduce_sum(
                            dum[:, 0:1], st_sb[:, 0, 0:4],
                            axis=mybir.AxisListType.X,
                        )
                nc.sync.dma_start(out[0, 0:128, 0:2], dum[:])

            def stream_pass():
                zq_tiles = []
                stats_sb = stpool.tile([f, b_pc * NCH, 2], F32, tag="stats")

                def mm_stage(m):
                    # Matmul stage for chunk m, emitted one chunk later so
                    # PE never waits at its queue head for fresh dT copies.
                    agg = agpool.tile([f, CH], F32, tag="agg")
                    nc.tensor.matmul(
                        agg[:], m["xn"][:, 2 * m["c"], :], p0_sb[:],
                        start=True, stop=False,
                    )
                    nc.tensor.matmul(
                        agg[:], m["xn"][:, 2 * m["c"] + 1, :], p1_sb[:],
                        start=False, stop=False,
                    )
                    for jt in range(NT):
                        nc.tensor.matmul(
                            agg[:],
                            m["xn"][:, jt, :],
                            m["dT"][:, jt, :],
                            start=False,
                            stop=(jt == NT - 1),
                        )
                    yT = yTpool.tile([f, CH], BF16, tag="yT")
                    nc.vector.tensor_copy(yT[:], agg[:])
                    zT = ztpool.tile([f, CH], F32, tag="zT")
                    nc.tensor.matmul(
                        zT[:], wt_sb[:], yT[:], start=True, stop=True
                    )
                    i0, gi = m["i0"], m["gi"]
                    nc.vector.tensor_copy(m["zq"][:, i0 : i0 + CH], zT[:])
                    nc.vector.reduce_sum(
                        stats_sb[:, gi, 0:1], zT[:], axis=mybir.AxisListType.X
                    )
                    sq = sqpool.tile([f, CH], F32, tag="sq")
                    nc.scalar.activation(
                        sq[:],
                        zT[:],
                        mybir.ActivationFunctionType.Square,
                        accum_out=stats_sb[:, gi, 1:2],
                    )

                pend = None
                for b in range(b_pc):
                    # --- per-batch node features (ACT queue) ---
                    xn_sb = xnpool.tile([128, NT, f], BF16, tag="xn")
                    nc.scalar.dma_start(
                        xn_sb[:], xn[b].rearrange("(t p) f -> p t f", p=128)
                    )
                    zq_sb = zqpool.tile([f, n], BF16, tag="zq")
                    zq_tiles.append(zq_sb)

                    for c in range(NCH):
                        i0 = c * CH
                        # --- ONE 4MB DMA: sim+dsim stripes for CH rows.
                        # Row interleave: chunk row (2p + r) lands on
                        # partition p, slot r — a 3-dim access pattern with
                        # 16KB contiguous reads per (partition, plane).
                        # Downstream free-axis positions within the chunk are
                        # pos = r*128 + q  <->  global row i0 + 2q + r; the
                        # host pre-permutes xt and the out store un-permutes.
                        st_sb = spool.tile([128, 2, RB * n], F32, tag="st")
                        nc.sync.dma_start(
                            st_sb[:],
                            edge[b, :, i0 : i0 + CH, :].rearrange(
                                "s (p r) n -> p s (r n)", r=RB
                            ),
                        )
                        # --- diff = sim - dsim (bf16) on GPSIMD: the only
                        # op gating stream-buffer release rides an engine
                        # with no PE/ACT-dependent work in its FIFO, so a
                        # lagging consumer can never stall the edge DMAs.
                        diff = dfpool.tile([128, RB * n], BF16, tag="diff")
                        nc.gpsimd.tensor_sub(
                            diff[:], st_sb[:, 0], st_sb[:, 1]
                        )

                        # --- transpose diff tiles: dT[j, i] = diff[i, j].
                        # Each PSUM tile holds TWO j-tiles; the PSUM->SBUF
                        # copies alternate DVE/ACT.
                        dT = dTpool.tile([128, NT, CH], BF16, tag="dT")
                        for jt2 in range(NT // 2):
                            tp = tppool.tile([128, 2, CH], BF16, tag="tp")
                            for k in range(2):
                                jt = 2 * jt2 + k
                                for r in range(RB):
                                    nc.tensor.transpose(
                                        tp[:, k, r * 128 : (r + 1) * 128],
                                        diff[
                                            :,
                                            r * n
                                            + jt * 128 : r * n
                                            + (jt + 1) * 128,
                                        ],
                                        i128_sb[:],
                                    )
                            if jt2 % 2 == 0:
                                nc.vector.tensor_copy(
                                    dT[:, 2 * jt2 : 2 * jt2 + 2, :], tp[:]
                                )
                            else:
                                nc.scalar.copy(
                                    dT[:, 2 * jt2 : 2 * jt2 + 2, :], tp[:]
                                )

                        # --- deferred matmul stage for the PREVIOUS chunk:
                        # yT[f,i] = x[i,f] + sum_j (x/N)[j,f] diff[i,j] via
                        # two constant permutation matmuls (p0/p1 hold 2048.0
                        # at (j, pos) where global j == row(pos), turning the
                        # xn (= x/2048) stationary back into exactly-bf16 x),
                        # then zT = W @ yT and BN partial sums.
                        if pend is not None:
                            mm_stage(pend)
                        pend = {
                            "xn": xn_sb,
                            "zq": zq_sb,
                            "dT": dT,
                            "c": c,
                            "i0": i0,
                            "gi": b * NCH + c,
                        }

                mm_stage(pend)

                # --- local stats -> launch sync-BN all-reduce (gpsimd) ---
                stats_loc = stpool.tile([f, 2], F32, tag="loc")
                nc.vector.reduce_sum(
                    stats_loc[:],
                    stats_sb[:].rearrange("p g s -> p s g"),
                    axis=mybir.AxisListType.X,
                )
                cc_in = drpool.tile([f, 2], F32, tag="cc_in")
                cc_out = drpool.tile([f, 2], F32, tag="cc_out")
                nc.scalar.dma_start(cc_in[:], stats_loc[:])
                if mode == "nocc":
                    nc.scalar.dma_start(cc_out[:], cc_in[:])
                else:
                    nc.gpsimd.collective_compute(
                        "AllReduce",
                        mybir.AluOpType.add,
                        replica_groups=[list(range(n_cores))],
                        ins=[cc_in.opt()],
                        outs=[cc_out.opt()],
                    )
                return {"zq": zq_tiles, "cc_out": cc_out}

            def tail_pass(st):
                stats_tot = bnpool.tile([f, 2], F32, tag="tot")
                nc.scalar.dma_start(stats_tot[:], st["cc_out"][:])

                # --- mean/var -> scale/shift ---
                sc_sb = bnpool.tile([f, 12], F32, tag="sc")
                mean = sc_sb[:, 0:1]
                es2 = sc_sb[:, 1:2]
                msq = sc_sb[:, 2:3]
                var = sc_sb[:, 3:4]
                std = sc_sb[:, 4:5]
                rstd = sc_sb[:, 5:6]
                scl = sc_sb[:, 6:7]
                tmp = sc_sb[:, 7:8]
                shf = sc_sb[:, 8:9]
                varp = sc_sb[:, 9:10]
                nc.vector.tensor_scalar_mul(mean, stats_tot[:, 0:1], inv_count)
                nc.vector.tensor_scalar_mul(es2, stats_tot[:, 1:2], inv_count)
                nc.vector.tensor_mul(msq, mean, mean)
                nc.vector.tensor_sub(var, es2, msq)
                nc.vector.tensor_scalar_add(varp, var, BN_EPS)
                nc.scalar.activation(std, varp, mybir.ActivationFunctionType.Sqrt)
                nc.vector.reciprocal(rstd, std)
                nc.vector.tensor_mul(scl, gamma_sb[:], rstd)
                nc.vector.tensor_mul(tmp, mean, scl)
                nc.vector.tensor_sub(shf, beta_sb[:], tmp)

                # --- apply BN+ReLU, untranspose, store ---
                for b in range(b_pc):
                    zr_sb = zrpool.tile([f, n], BF16, tag="zr")
                    nc.scalar.activation(
                        zr_sb[:],
                        st["zq"][b][:],
                        mybir.ActivationFunctionType.Relu,
                        bias=shf,
                        scale=scl,
                    )
                    out_sb = outpool.tile([128, NCH, RB * f], F32, tag="out")
                    for ct in range(NT):
                        bp = bppool.tile([128, f], BF16, tag="bp")
                        nc.tensor.transpose(
                            bp[:], zr_sb[:, ct * 128 : (ct + 1) * 128], i64_sb[:]
                        )
                        nc.vector.tensor_copy(
                            out_sb[
                                :, ct // RB, (ct % RB) * f : (ct % RB + 1) * f
                            ],
                            bp[:],
                        )
                    nc.scalar.dma_start(
                        out[b].rearrange("(c q r) f -> q c (r f)", q=128, r=RB),
                        out_sb[:],
                    )

            def cc_only_pass(var, ccsh):
                # isolate the per-pass collective cost (no edge stream)
                loc = stpool.tile([f, 2], F32, tag="cloc")
                nc.vector.tensor_scalar_mul(loc[:, 0:1], gamma_sb[:], 2.0)
                nc.vector.tensor_scalar_mul(loc[:, 1:2], gamma_sb[:], 3.0)
                cc_in = drpool.tile([f, 2], F32, tag="cc_in")
                nc.gpsimd.dma_start(cc_in[:], loc[:])
                groups = [list(range(n_cores))]
                if var == "ag":
                    cc_out = drpool.tile([n_cores, f, 2], F32, tag="cc_oag")
                    nc.gpsimd.collective_compute(
                        "AllGather", mybir.AluOpType.bypass,
                        replica_groups=groups,
                        ins=[cc_in.opt()], outs=[cc_out.opt()],
                    )
                    tot8 = bnpool.tile([f, n_cores, 2], F32, tag="tot8")
                    nc.gpsimd.dma_start(
                        tot8[:], cc_out.rearrange("g p s -> p g s")
                    )
                    tot = bnpool.tile([f, 2], F32, tag="ctot")
                    nc.vector.reduce_sum(
                        tot[:], tot8[:].rearrange("p g s -> p s g"),
                        axis=mybir.AxisListType.X,
                    )
                else:
                    if var == "sh":
                        cc_out = ccsh
                    else:
                        cc_out = drpool.tile([f, 2], F32, tag="cc_out")
                    nc.gpsimd.collective_compute(
                        "AllReduce", mybir.AluOpType.add,
                        replica_groups=groups,
                        ins=[cc_in.opt()], outs=[cc_out.opt()],
                    )
                    tot = bnpool.tile([f, 2], F32, tag="ctot")
                    nc.gpsimd.dma_start(tot[:], cc_out[:])
                dum = cpool.tile([f, 2], F32, tag="cdum")
                nc.vector.tensor_copy(dum[:], tot[:])
                nc.gpsimd.dma_start(out[0, 0:f, 0:2], dum[:])

            if mode.startswith("cconly"):
                var = mode.split("-")[1]
                ccshs = [
                    nc.dram_tensor(
                        f"ccsh{i}", [f, 2], F32,
                        kind="Internal", addr_space="Shared",
                    ).ap()
                    for i in range(reps)
                ] if var == "sh" else [None] * reps
                for i in range(reps):
                    cc_only_pass(var, ccshs[i])
            elif mode in ("dmaonly", "dmacast"):
                for _ in range(reps):
                    dma_only_pass(cast=(mode == "dmacast"))
            else:
                # Software-pipeline the tail TWO passes behind the stream:
                # the sync-BN AllReduce of pass p is consumed only after
                # stream(p+2), so cross-core skew up to ~2 passes never
                # stalls any engine (the collective is a rendezvous; slack
                # amortizes worst-core jitter).
                DEFER = 2
                pend = []
                for _ in range(reps):
                    pend.append(stream_pass())
                    if len(pend) > DEFER:
                        tail_pass(pend.pop(0))
                for st in pend:
                    tail_pass(st)

    nc.compile()
    return nc


def make_in_maps(node_feats, edge_feats, W, gamma, beta, n_cores=N_CORES):
    b, n, f = node_feats.shape
    b_pc = b // n_cores
    node_feats = np.asarray(node_feats, dtype=np.float32)
    edge_feats = np.asarray(edge_feats, dtype=np.float32)
    wt = np.ascontiguousarray(
        np.asarray(W, dtype=np.float32).T
    ).astype(ml_dtypes.bfloat16)
    gamma = np.asarray(gamma, dtype=np.float32).reshape(f, 1)
    beta = np.asarray(beta, dtype=np.float32).reshape(f, 1)
    i128 = np.eye(128, dtype=np.float32).astype(ml_dtypes.bfloat16)
    i64 = np.eye(f, dtype=np.float32).astype(ml_dtypes.bfloat16)
    in_maps = []
    # Residual-permutation constants: chunk-local position pos = r*128 + q
    # holds global row i = 2q + r; p0/p1 put 2048.0 at (j_within_tile, pos)
    # where the chunk's global j equals that row (j-tile 0 / 1 of the chunk).
    CH, RB = 256, 2
    p0 = np.zeros((128, CH), dtype=np.float32)
    p1 = np.zeros((128, CH), dtype=np.float32)
    for pos in range(CH):
        r, q = divmod(pos, 128)
        i_loc = 2 * q + r
        if i_loc < 128:
            p0[i_loc, pos] = 2048.0
        else:
            p1[i_loc - 128, pos] = 2048.0
    p0 = p0.astype(ml_dtypes.bfloat16)
    p1 = p1.astype(ml_dtypes.bfloat16)
    for c in range(n_cores):
        sl = slice(c * b_pc, (c + 1) * b_pc)
        xs = node_feats[sl]
        in_maps.append(
            {
                "edge": edge_feats[sl],
                "p0": p0,
                "p1": p1,
                "xn": (xs / np.float32(n)).astype(ml_dtypes.bfloat16),
                "wt": wt,
                "gamma": gamma,
                "beta": beta,
                "i128": i128,
                "i64": i64,
            }
        )
    return in_maps


_NC_CACHE = {}


def _get_nc(key=(N_CORES, B_PC, N, F)):
    if key not in _NC_CACHE:
        _NC_CACHE[key] = build_nc(*key)
    return _NC_CACHE[key]


def kernel(node_feats, edge_feats, W, gamma, beta):
    node_feats = np.asarray(node_feats)
    edge_feats = np.asarray(edge_feats)
    b, n, f = node_feats.shape
    n_cores = N_CORES
    b_pc = b // n_cores
    nc = _get_nc((n_cores, b_pc, n, f))
    in_maps = make_in_maps(node_feats, edge_feats, W, gamma, beta, n_cores)
    res = run_bass_kernel_spmd(nc, in_maps, list(range(n_cores)))
    outs = [res.results[c]["out"] for c in range(n_cores)]
    return np.concatenate(outs, axis=0).astype(np.float32)
